# revision 13
# baseline (speedup 1.0000x reference)
"""ALNN variant kernel for 8 TRN2 NeuronCores (pure data-parallel over batch).

Math (per batch b, ref-time k; rt_k = linspace(0,48,49) = k):
  e   = exp(-relu(alpha_k) * |T - k|)
  s1  = relu(X*wt0 + relu(X)*e*wt1 + M*wt2 + PD*wt3 + 4*bt)
  out = relu(sum_l s1*wv + 200*bv)      -> [B, K, D]

Raw-bass implementation (this walrus build allows only ONE attached sync-wait
per instruction, so Tile's multi-wait output doesn't compile; explicit
standalone wait_ge instructions are used instead).

Layout: partitions = l (chunks 128/72), free = (kblk=7, b=8, d=36).
ScalarE: u = Abs(ra_k*T - ra_k*k) via per-partition scale/bias; e = Exp(-u).
VectorE: bf16 products/sums, stride-0 broadcast APs; relu+wv fused via
scalar_tensor_tensor. TensorE: l-reduction via one-hot-window matmuls
accumulating a PSUM [50, b*d] tile across all 98 matmuls.
Pipeline: 14 stages (7 k-blocks x 2 l-chunks), e/z double-buffered,
ACT -> DVE -> PE chained with asem/vsem/psem.
"""

import numpy as np
import ml_dtypes
from contextlib import ExitStack

import concourse.bass as bass
import concourse.mybir as mybir
from concourse.bass_utils import run_bass_kernel_spmd

B, K, L, D = 64, 49, 200, 36
NCORES = 8
BL = B // NCORES
KB = 7
NSTAGE = (K // KB) * 2
CHUNKS = [(0, 128), (128, 72)]
BF16 = mybir.dt.bfloat16
F32 = mybir.dt.float32
AF = mybir.ActivationFunctionType
ALU = mybir.AluOpType

_NC_CACHE = {}


def _ap(handle_ap, dims, extra_offset=0):
    """Rebuild an AP with an explicit [stride, n] dim list."""
    return bass.AP(handle_ap.tensor, handle_ap.offset + extra_offset, dims)


def build_nc():
    nc = bass.Bass()
    T_e = nc.declare_dram_parameter("T", [BL, L, D], F32, isOutput=False)
    X_e = nc.declare_dram_parameter("X", [BL, L, D], BF16, isOutput=False)
    RX_e = nc.declare_dram_parameter("RX", [BL, L, D], BF16, isOutput=False)
    M_e = nc.declare_dram_parameter("M", [BL, L, D], BF16, isOutput=False)
    PD_e = nc.declare_dram_parameter("PD", [BL, L, D], BF16, isOutput=False)
    W_es = [nc.declare_dram_parameter(f"W{c}", [K, L, D], BF16, isOutput=False) for c in range(4)]
    BT4_e = nc.declare_dram_parameter("BT4", [K, L, D], BF16, isOutput=False)
    WV_e = nc.declare_dram_parameter("WV", [K, L, D], BF16, isOutput=False)
    AB_e = nc.declare_dram_parameter("AB", [128, 2 * K], F32, isOutput=False)
    BV_e = nc.declare_dram_parameter("BV", [K, D], F32, isOutput=False)
    ARR_e = nc.declare_dram_parameter("ARR", [128, 200], BF16, isOutput=False)
    OUT_e = nc.declare_dram_parameter("out", [BL, K, D], F32, isOutput=True)

    es = ExitStack()
    with es:
        sb = lambda name, shape, dt: es.enter_context(nc.sbuf_tensor(name, shape, dt))
        ab = sb("ab", [128, 2 * K], F32)
        arr = sb("arr", [128, 200], BF16)
        bvt = sb("bvt", [K, D], F32)
        Tt, Xt, RXt, Mt, PDt = {}, {}, {}, {}, {}
        Wt = {}  # (c, ci) -> resident weight tensor [P, K, D]
        for ci, (l0, P) in enumerate(CHUNKS):
            Tt[ci] = sb(f"T{ci}", [P, BL, D], F32)
            Xt[ci] = sb(f"X{ci}", [P, BL, D], BF16)
            RXt[ci] = sb(f"RX{ci}", [P, BL, D], BF16)
            Mt[ci] = sb(f"M{ci}", [P, BL, D], BF16)
            PDt[ci] = sb(f"PD{ci}", [P, BL, D], BF16)
            for c in range(6):
                Wt[(c, ci)] = sb(f"w{c}_{ci}", [P, K, D], BF16)
        eb = [sb(f"e{i}", [128, KB, BL, D], BF16) for i in range(2)]
        zb = [sb(f"z{i}", [128, KB, BL, D], BF16) for i in range(2)]
        ut = sb("ut", [128, BL, D], F32)
        It = sb("It", [128, KB, BL, D], BF16)
        t1 = sb("t1", [128, KB, BL, D], BF16)
        t2 = sb("t2", [128, KB, BL, D], BF16)
        t3 = sb("t3", [128, KB, BL, D], BF16)
        t4 = sb("t4", [128, KB, BL, D], BF16)
        ot = sb("ot", [K, BL * D], F32)
        ot2 = sb("ot2", [K, BL * D], F32)
        pt = es.enter_context(nc.psum_tensor("acc", [50, BL * D], F32))

        NDMA = 3 + 2 * 5 + 2 * 6  # consts + data + weights
        with (
            nc.Block() as block,
            nc.semaphore("dsem") as dsem,
            nc.semaphore("asem") as asem,
            nc.semaphore("vsem") as vsem,
            nc.semaphore("psem") as psem,
        ):
            @block.gpsimd
            def _(g):
                g.dma_start(out=ab[:], in_=AB_e[:]).then_inc(dsem, 16)
                g.dma_start(out=arr[:], in_=ARR_e[:]).then_inc(dsem, 16)
                g.dma_start(out=bvt[:], in_=BV_e[:]).then_inc(dsem, 16)
                for ci, (l0, P) in enumerate(CHUNKS):
                    for tile, ext in ((Tt[ci], T_e), (Xt[ci], X_e), (RXt[ci], RX_e),
                                      (Mt[ci], M_e), (PDt[ci], PD_e)):
                        g.dma_start(
                            out=tile[:],
                            in_=ext[:, l0 : l0 + P, :].rearrange("b l d -> l b d"),
                        ).then_inc(dsem, 16)
                    for c in range(6):
                        ext = (W_es + [BT4_e, WV_e])[c]
                        g.dma_start(
                            out=Wt[(c, ci)][:],
                            in_=ext[:, l0 : l0 + P, :].rearrange("k l d -> l k d"),
                        ).then_inc(dsem, 16)
                # output
                g.wait_ge(asem, NSTAGE + 1)
                oh = ot2[:]
                o3 = _ap(oh, [oh.ap[0], [D, BL], [1, D]])
                g.dma_start(
                    out=OUT_e[:].rearrange("b k d -> k b d"), in_=o3
                ).then_inc(dsem, 16)
                g.wait_ge(dsem, 16 * (NDMA + 1))

            @block.scalar
            def _(a):
                a.wait_ge(dsem, 16 * NDMA)
                s = 0
                for kb in range(K // KB):
                    k0 = kb * KB
                    for ci, (l0, P) in enumerate(CHUNKS):
                        if s >= 2:
                            a.wait_ge(vsem, s - 1)
                        e = eb[s % 2]
                        for ki in range(KB):
                            k = k0 + ki
                            a.activation(
                                ut[0:P], Tt[ci][:], AF.Abs,
                                bias=ab[0:P, K + k : K + k + 1],
                                scale=ab[0:P, k : k + 1],
                            )
                            ins = a.activation(e[0:P, ki], ut[0:P], AF.Exp, scale=-1.0)
                        ins.then_inc(asem, 1)
                        s += 1
                a.wait_ge(vsem, NSTAGE + 1)
                a.activation(ot2[:], ot[:], AF.Relu).then_inc(asem, 1)

            @block.vector
            def _(v):
                v.wait_ge(dsem, 16 * NDMA)

                def bc_w(c, ci, k0, P):  # weight [P,K,D] slice -> [P,KB,(BL:0),D]
                    h = Wt[(c, ci)][:, k0 : k0 + KB, :]
                    return _ap(h, [h.ap[0], h.ap[1], [0, BL], h.ap[2]])

                def bc_d(t):  # data [P,BL,D] -> [P,(KB:0),BL,D]
                    h = t[:]
                    return _ap(h, [h.ap[0], [0, KB], h.ap[1], h.ap[2]])

                s = 0
                for kb in range(K // KB):
                    k0 = kb * KB
                    for ci, (l0, P) in enumerate(CHUNKS):
                        v.wait_ge(asem, s + 1)
                        if s >= 2:
                            v.wait_ge(psem, s - 1)
                        e, z = eb[s % 2], zb[s % 2]
                        v.tensor_tensor(It[0:P], e[0:P], bc_d(RXt[ci])[0:P] if False else _ap(bc_d(RXt[ci]), [[RXt[ci][:].ap[0][0], P], [0, KB], RXt[ci][:].ap[1], RXt[ci][:].ap[2]]), ALU.mult)
                        v.tensor_tensor(t1[0:P], It[0:P], bc_w(1, ci, k0, P), ALU.mult)
                        v.tensor_tensor(t2[0:P], _ap(bc_d(Xt[ci]), [[Xt[ci][:].ap[0][0], P], [0, KB], Xt[ci][:].ap[1], Xt[ci][:].ap[2]]), bc_w(0, ci, k0, P), ALU.mult)
                        v.tensor_tensor(t1[0:P], t1[0:P], t2[0:P], ALU.add)
                        v.tensor_tensor(t3[0:P], _ap(bc_d(Mt[ci]), [[Mt[ci][:].ap[0][0], P], [0, KB], Mt[ci][:].ap[1], Mt[ci][:].ap[2]]), bc_w(2, ci, k0, P), ALU.mult)
                        v.tensor_tensor(t4[0:P], _ap(bc_d(PDt[ci]), [[PDt[ci][:].ap[0][0], P], [0, KB], PDt[ci][:].ap[1], PDt[ci][:].ap[2]]), bc_w(3, ci, k0, P), ALU.mult)
                        v.tensor_tensor(t3[0:P], t3[0:P], t4[0:P], ALU.add)
                        v.tensor_tensor(t1[0:P], t1[0:P], t3[0:P], ALU.add)
                        v.tensor_tensor(t1[0:P], t1[0:P], bc_w(4, ci, k0, P), ALU.add)
                        wv = Wt[(5, ci)][:, k0 : k0 + KB, :]
                        for ki in range(KB):
                            wvk = _ap(wv, [wv.ap[0], [0, BL], [1, D]], extra_offset=ki * D)
                            ins = v.scalar_tensor_tensor(
                                z[0:P, ki], t1[0:P, ki], 0.0, wvk, ALU.max, ALU.mult
                            )
                        ins.then_inc(vsem, 1)
                        s += 1
                v.wait_ge(psem, NSTAGE)
                bvh = bvt[:]
                bvb = _ap(bvh, [bvh.ap[0], [0, BL], bvh.ap[1]])
                ph = pt[0:K]
                p3 = _ap(ph, [ph.ap[0], [D, BL], [1, D]])
                oh = ot[:]
                o3 = _ap(oh, [oh.ap[0], [D, BL], [1, D]])
                v.tensor_tensor(o3, p3, bvb, ALU.add).then_inc(vsem, 1)

            @block.tensor
            def _(pe):
                s = 0
                mm = 0
                for kb in range(K // KB):
                    k0 = kb * KB
                    for ci, (l0, P) in enumerate(CHUNKS):
                        pe.wait_ge(vsem, s + 1)
                        z = zb[s % 2]
                        for ki in range(KB):
                            k = k0 + ki
                            mm += 1
                            s0 = (48 - k) if k % 2 == 0 else (149 - k)
                            zr = z[0:P, ki]
                            zr2 = _ap(zr, [zr.ap[0], [1, BL * D]])
                            ins = pe.matmul(
                                pt[:], arr[0:P, s0 : s0 + 50], zr2,
                                start=(mm == 1), stop=(mm == NSTAGE * KB),
                                skip_group_check=True,
                            )
                        ins.then_inc(psem, 1)
                        s += 1

        nc.finalize()
    return nc


def kernel(X, T, M, PD, alpha, w_v, w_t, b_v, b_t, _trace=False):
    bf16 = ml_dtypes.bfloat16
    X = np.asarray(X, np.float32); T = np.asarray(T, np.float32)
    M = np.asarray(M, np.float32); PD = np.asarray(PD, np.float32)
    ra = np.maximum(np.asarray(alpha, np.float32).reshape(K), 0.0)
    rt = np.arange(K, dtype=np.float32)
    AB = np.tile(np.concatenate([ra, -ra * rt])[None, :], (128, 1)).astype(np.float32)
    Ws = [np.ascontiguousarray(w_t[..., c]).astype(bf16) for c in range(4)]
    BT4 = (4.0 * b_t[..., 0]).astype(bf16)
    WV = np.asarray(w_v).astype(bf16)
    BV = (float(L) * np.asarray(b_v)[:, 0, :]).astype(np.float32)
    ARR = np.zeros((128, 200), dtype=bf16)
    ARR[:, 48] = 1.0
    ARR[:, 149] = 1.0
    Xb = X.astype(bf16)
    RXb = np.maximum(X, 0.0).astype(bf16)
    Mb = M.astype(bf16)
    PDb = PD.astype(bf16)

    shared = {"W0": Ws[0], "W1": Ws[1], "W2": Ws[2], "W3": Ws[3], "BT4": BT4,
              "WV": WV, "AB": AB, "BV": BV, "ARR": ARR}
    in_maps = []
    for i in range(NCORES):
        s = slice(i * BL, (i + 1) * BL)
        m = {"T": np.ascontiguousarray(T[s]), "X": np.ascontiguousarray(Xb[s]),
             "RX": np.ascontiguousarray(RXb[s]), "M": np.ascontiguousarray(Mb[s]),
             "PD": np.ascontiguousarray(PDb[s])}
        m.update(shared)
        in_maps.append(m)

    if "nc" not in _NC_CACHE:
        _NC_CACHE["nc"] = build_nc()
    nc = _NC_CACHE["nc"]
    res = run_bass_kernel_spmd(nc, in_maps, list(range(NCORES)), trace=_trace)
    out = np.concatenate([np.asarray(res.results[i]["out"]) for i in range(NCORES)], 0)
    if _trace:
        kernel._last_result = res
    return out.astype(np.float32)


# revision 14
# speedup vs baseline: 1.0694x; 1.0694x over previous
"""ALNN variant kernel for 8 TRN2 NeuronCores (pure data-parallel over batch).

Math (per batch b, ref-time k; rt_k = linspace(0,48,49) = k):
  e   = exp(-relu(alpha_k) * |T - k|)
  s1  = relu(X*wt0 + relu(X)*e*wt1 + M*wt2 + PD*wt3 + 4*bt)
  out = relu(sum_l s1*wv + 200*bv)      -> [B, K, D]

Raw-bass implementation (this walrus build allows only ONE attached sync-wait
per instruction, so Tile's multi-wait output doesn't compile; explicit
standalone wait_ge instructions are used instead).

Layout: partitions = l (chunks 128/72), free = (kblk=7, b=8, d=36).
ScalarE: u = Abs(ra_k*T - ra_k*k) via per-partition scale/bias; e = Exp(-u).
VectorE: bf16 products/sums, stride-0 broadcast APs; relu+wv fused via
scalar_tensor_tensor. TensorE: l-reduction via one-hot-window matmuls
accumulating a PSUM [50, b*d] tile across all 98 matmuls.
Pipeline: 14 stages (7 k-blocks x 2 l-chunks), e/z double-buffered,
ACT -> DVE -> PE chained with asem/vsem/psem.
"""

import numpy as np
import ml_dtypes
from contextlib import ExitStack

import concourse.bass as bass
import concourse.mybir as mybir
from concourse.bass_utils import run_bass_kernel_spmd

B, K, L, D = 64, 49, 200, 36
NCORES = 8
BL = B // NCORES
KB = 7
NSTAGE = (K // KB) * 2
CHUNKS = [(0, 128), (128, 72)]
BF16 = mybir.dt.bfloat16
F32 = mybir.dt.float32
AF = mybir.ActivationFunctionType
ALU = mybir.AluOpType

_NC_CACHE = {}


def _ap(handle_ap, dims, extra_offset=0):
    """Rebuild an AP with an explicit [stride, n] dim list."""
    return bass.AP(handle_ap.tensor, handle_ap.offset + extra_offset, dims)


def build_nc():
    nc = bass.Bass()
    T_e = nc.declare_dram_parameter("T", [BL, L, D], F32, isOutput=False)
    X_e = nc.declare_dram_parameter("X", [BL, L, D], BF16, isOutput=False)
    RX_e = nc.declare_dram_parameter("RX", [BL, L, D], BF16, isOutput=False)
    M_e = nc.declare_dram_parameter("M", [BL, L, D], BF16, isOutput=False)
    PD_e = nc.declare_dram_parameter("PD", [BL, L, D], BF16, isOutput=False)
    W_es = [nc.declare_dram_parameter(f"W{c}", [K, L, D], BF16, isOutput=False) for c in range(4)]
    BT4_e = nc.declare_dram_parameter("BT4", [K, L, D], BF16, isOutput=False)
    WV_e = nc.declare_dram_parameter("WV", [K, L, D], BF16, isOutput=False)
    AB_e = nc.declare_dram_parameter("AB", [128, 2 * K], F32, isOutput=False)
    BV_e = nc.declare_dram_parameter("BV", [K, D], F32, isOutput=False)
    ARR_e = nc.declare_dram_parameter("ARR", [128, 200], BF16, isOutput=False)
    OUT_e = nc.declare_dram_parameter("out", [BL, K, D], F32, isOutput=True)

    es = ExitStack()
    with es:
        sb = lambda name, shape, dt: es.enter_context(nc.sbuf_tensor(name, shape, dt))
        ab = sb("ab", [128, 2 * K], F32)
        arr = sb("arr", [128, 200], BF16)
        bvt = sb("bvt", [K, D], F32)
        Tt, Xt, RXt, Mt, PDt = {}, {}, {}, {}, {}
        Wt = {}  # (c, ci) -> resident weight tensor [P, K, D]
        for ci, (l0, P) in enumerate(CHUNKS):
            Tt[ci] = sb(f"T{ci}", [P, BL, D], F32)
            Xt[ci] = sb(f"X{ci}", [P, BL, D], BF16)
            RXt[ci] = sb(f"RX{ci}", [P, BL, D], BF16)
            Mt[ci] = sb(f"M{ci}", [P, BL, D], BF16)
            PDt[ci] = sb(f"PD{ci}", [P, BL, D], BF16)
            for c in range(6):
                Wt[(c, ci)] = sb(f"w{c}_{ci}", [P, K, D], BF16)
        eb = [sb(f"e{i}", [128, KB, BL, D], BF16) for i in range(2)]
        zb = [sb(f"z{i}", [128, KB, BL, D], BF16) for i in range(2)]
        ut = sb("ut", [128, BL, D], F32)
        It = sb("It", [128, KB, BL, D], BF16)
        t1 = sb("t1", [128, KB, BL, D], BF16)
        t2 = sb("t2", [128, KB, BL, D], BF16)
        t3 = sb("t3", [128, KB, BL, D], BF16)
        t4 = sb("t4", [128, KB, BL, D], BF16)
        ot = sb("ot", [K, BL * D], F32)
        ot2 = sb("ot2", [K, BL * D], F32)
        pt = es.enter_context(nc.psum_tensor("acc", [50, BL * D], F32))

        NDMA = 3 + 2 * 5 + 2 * 6  # consts + data + weights
        with (
            nc.Block() as block,
            nc.semaphore("dsem") as dsem,
            nc.semaphore("asem") as asem,
            nc.semaphore("vsem") as vsem,
            nc.semaphore("psem") as psem,
        ):
            @block.gpsimd
            def _(g):
                g.dma_start(out=ab[:], in_=AB_e[:]).then_inc(dsem, 16)
                g.dma_start(out=arr[:], in_=ARR_e[:]).then_inc(dsem, 16)
                for ci, (l0, P) in enumerate(CHUNKS):
                    for tile, ext in ((Tt[ci], T_e), (Xt[ci], X_e), (RXt[ci], RX_e),
                                      (Mt[ci], M_e), (PDt[ci], PD_e)):
                        g.dma_start(
                            out=tile[:],
                            in_=ext[:, l0 : l0 + P, :].rearrange("b l d -> l b d"),
                        ).then_inc(dsem, 16)
                    for c in range(6):
                        ext = (W_es + [BT4_e, WV_e])[c]
                        g.dma_start(
                            out=Wt[(c, ci)][:],
                            in_=ext[:, l0 : l0 + P, :].rearrange("k l d -> l k d"),
                        ).then_inc(dsem, 16)
                g.dma_start(out=bvt[:], in_=BV_e[:]).then_inc(dsem, 16)
                # output
                g.wait_ge(asem, NSTAGE + 1)
                oh = ot2[:]
                o3 = _ap(oh, [oh.ap[0], [D, BL], [1, D]])
                g.dma_start(
                    out=OUT_e[:].rearrange("b k d -> k b d"), in_=o3
                ).then_inc(dsem, 16)
                g.wait_ge(dsem, 16 * (NDMA + 1))

            @block.scalar
            def _(a):
                s = 0
                for kb in range(K // KB):
                    k0 = kb * KB
                    for ci, (l0, P) in enumerate(CHUNKS):
                        if s == 0:
                            a.wait_ge(dsem, 16 * 3)    # ab + T0 loaded
                        elif s == 1:
                            a.wait_ge(dsem, 16 * 14)   # T1 loaded
                        if s >= 2:
                            a.wait_ge(vsem, s - 1)
                        e = eb[s % 2]
                        for ki in range(KB):
                            k = k0 + ki
                            a.activation(
                                ut[0:P], Tt[ci][:], AF.Abs,
                                bias=ab[0:P, K + k : K + k + 1],
                                scale=ab[0:P, k : k + 1],
                            )
                            ins = a.activation(e[0:P, ki], ut[0:P], AF.Exp, scale=-1.0)
                        ins.then_inc(asem, 1)
                        s += 1
                a.wait_ge(vsem, NSTAGE + 1)
                a.activation(ot2[:], ot[:], AF.Relu).then_inc(asem, 1)

            @block.vector
            def _(v):

                def bc_w(c, ci, k0, P):  # weight [P,K,D] slice -> [P,KB,(BL:0),D]
                    h = Wt[(c, ci)][:, k0 : k0 + KB, :]
                    return _ap(h, [h.ap[0], h.ap[1], [0, BL], h.ap[2]])

                def bc_d(t):  # data [P,BL,D] -> [P,(KB:0),BL,D]
                    h = t[:]
                    return _ap(h, [h.ap[0], [0, KB], h.ap[1], h.ap[2]])

                s = 0
                for kb in range(K // KB):
                    k0 = kb * KB
                    for ci, (l0, P) in enumerate(CHUNKS):
                        if s == 0:
                            v.wait_ge(dsem, 16 * 13)   # chunk-0 data+weights
                        elif s == 1:
                            v.wait_ge(dsem, 16 * 24)   # chunk-1 data+weights
                        v.wait_ge(asem, s + 1)
                        if s >= 2:
                            v.wait_ge(psem, s - 1)
                        e, z = eb[s % 2], zb[s % 2]
                        v.tensor_tensor(It[0:P], e[0:P], bc_d(RXt[ci])[0:P] if False else _ap(bc_d(RXt[ci]), [[RXt[ci][:].ap[0][0], P], [0, KB], RXt[ci][:].ap[1], RXt[ci][:].ap[2]]), ALU.mult)
                        v.tensor_tensor(t1[0:P], It[0:P], bc_w(1, ci, k0, P), ALU.mult)
                        v.tensor_tensor(t2[0:P], _ap(bc_d(Xt[ci]), [[Xt[ci][:].ap[0][0], P], [0, KB], Xt[ci][:].ap[1], Xt[ci][:].ap[2]]), bc_w(0, ci, k0, P), ALU.mult)
                        v.tensor_tensor(t1[0:P], t1[0:P], t2[0:P], ALU.add)
                        v.tensor_tensor(t3[0:P], _ap(bc_d(Mt[ci]), [[Mt[ci][:].ap[0][0], P], [0, KB], Mt[ci][:].ap[1], Mt[ci][:].ap[2]]), bc_w(2, ci, k0, P), ALU.mult)
                        v.tensor_tensor(t4[0:P], _ap(bc_d(PDt[ci]), [[PDt[ci][:].ap[0][0], P], [0, KB], PDt[ci][:].ap[1], PDt[ci][:].ap[2]]), bc_w(3, ci, k0, P), ALU.mult)
                        v.tensor_tensor(t3[0:P], t3[0:P], t4[0:P], ALU.add)
                        v.tensor_tensor(t1[0:P], t1[0:P], t3[0:P], ALU.add)
                        v.tensor_tensor(t1[0:P], t1[0:P], bc_w(4, ci, k0, P), ALU.add)
                        wv = Wt[(5, ci)][:, k0 : k0 + KB, :]
                        for ki in range(KB):
                            wvk = _ap(wv, [wv.ap[0], [0, BL], [1, D]], extra_offset=ki * D)
                            ins = v.scalar_tensor_tensor(
                                z[0:P, ki], t1[0:P, ki], 0.0, wvk, ALU.max, ALU.mult
                            )
                        ins.then_inc(vsem, 1)
                        s += 1
                v.wait_ge(dsem, 16 * 25)           # bv loaded
                v.wait_ge(psem, NSTAGE)
                bvh = bvt[:]
                bvb = _ap(bvh, [bvh.ap[0], [0, BL], bvh.ap[1]])
                ph = pt[0:K]
                p3 = _ap(ph, [ph.ap[0], [D, BL], [1, D]])
                oh = ot[:]
                o3 = _ap(oh, [oh.ap[0], [D, BL], [1, D]])
                v.tensor_tensor(o3, p3, bvb, ALU.add).then_inc(vsem, 1)

            @block.tensor
            def _(pe):
                s = 0
                mm = 0
                for kb in range(K // KB):
                    k0 = kb * KB
                    for ci, (l0, P) in enumerate(CHUNKS):
                        pe.wait_ge(vsem, s + 1)
                        z = zb[s % 2]
                        for ki in range(KB):
                            k = k0 + ki
                            mm += 1
                            s0 = (48 - k) if k % 2 == 0 else (149 - k)
                            zr = z[0:P, ki]
                            zr2 = _ap(zr, [zr.ap[0], [1, BL * D]])
                            ins = pe.matmul(
                                pt[:], arr[0:P, s0 : s0 + 50], zr2,
                                start=(mm == 1), stop=(mm == NSTAGE * KB),
                                skip_group_check=True,
                            )
                        ins.then_inc(psem, 1)
                        s += 1

        nc.finalize()
    return nc


def kernel(X, T, M, PD, alpha, w_v, w_t, b_v, b_t, _trace=False):
    bf16 = ml_dtypes.bfloat16
    X = np.asarray(X, np.float32); T = np.asarray(T, np.float32)
    M = np.asarray(M, np.float32); PD = np.asarray(PD, np.float32)
    ra = np.maximum(np.asarray(alpha, np.float32).reshape(K), 0.0)
    rt = np.arange(K, dtype=np.float32)
    AB = np.tile(np.concatenate([ra, -ra * rt])[None, :], (128, 1)).astype(np.float32)
    Ws = [np.ascontiguousarray(w_t[..., c]).astype(bf16) for c in range(4)]
    BT4 = (4.0 * b_t[..., 0]).astype(bf16)
    WV = np.asarray(w_v).astype(bf16)
    BV = (float(L) * np.asarray(b_v)[:, 0, :]).astype(np.float32)
    ARR = np.zeros((128, 200), dtype=bf16)
    ARR[:, 48] = 1.0
    ARR[:, 149] = 1.0
    Xb = X.astype(bf16)
    RXb = np.maximum(X, 0.0).astype(bf16)
    Mb = M.astype(bf16)
    PDb = PD.astype(bf16)

    shared = {"W0": Ws[0], "W1": Ws[1], "W2": Ws[2], "W3": Ws[3], "BT4": BT4,
              "WV": WV, "AB": AB, "BV": BV, "ARR": ARR}
    in_maps = []
    for i in range(NCORES):
        s = slice(i * BL, (i + 1) * BL)
        m = {"T": np.ascontiguousarray(T[s]), "X": np.ascontiguousarray(Xb[s]),
             "RX": np.ascontiguousarray(RXb[s]), "M": np.ascontiguousarray(Mb[s]),
             "PD": np.ascontiguousarray(PDb[s])}
        m.update(shared)
        in_maps.append(m)

    if "nc" not in _NC_CACHE:
        _NC_CACHE["nc"] = build_nc()
    nc = _NC_CACHE["nc"]
    res = run_bass_kernel_spmd(nc, in_maps, list(range(NCORES)), trace=_trace)
    out = np.concatenate([np.asarray(res.results[i]["out"]) for i in range(NCORES)], 0)
    if _trace:
        kernel._last_result = res
    return out.astype(np.float32)
